# revision 3
# baseline (speedup 1.0000x reference)
"""Trainium2 Bass kernel for nn_CrossAtten: cross-attention
out = softmax((q Wq^T)(kv Wk^T)^T / sqrt(D)) @ (kv Wv^T) @ Wout^T + bout

Shapes (hardcoded): q,kv [4,16,2048,128] fp32; Wq,Wout [128,128]; Wkv [256,128]; bout [128].
Sharding: batch*heads (64 pairs) split 8 per NeuronCore across 8 cores (pure data parallel).

Algebraic restructure (host-side weight folding):
  A    = Wq^T @ Wk          -> scores S = q A kv^T     (one projected tensor u = qA)
  WvoT = Wv^T @ Wout^T      -> PV matmul directly yields final projection (pre-bias)
Softmax is computed max-free (logits ~ N(0,1), |logit| < ~7, exp is safe in fp32),
with scores produced transposed [j, i] so the PV contraction (over j) needs no
transpose of the attention matrix; denominators come from a ones-vector matmul.
PE matmuls run in float32r (TF32-class: ~1.6e-4 rel err, full 1 cycle/row speed).
"""
import sys

if "/opt/trn_rl_repo" not in sys.path:
    sys.path.insert(0, "/opt/trn_rl_repo")

from contextlib import ExitStack

import numpy as np

import concourse.bacc as bacc
import concourse.tile as tile
import concourse.mybir as mybir
from concourse.bass_utils import run_bass_kernel_spmd

B, H, I, J, D = 4, 16, 2048, 2048, 128
BH = B * H
N_CORES = 8
PER_CORE = BH // N_CORES          # 8 (b,h) pairs per core
P = 128                           # partitions
IT = I // P                       # 16 i-tiles
JT = J // P                       # 16 j-tiles
IC = 512                          # i-chunk (columns per scores/PV matmul)
NIC = I // IC                     # 4 i-chunks
SCALE = D ** -0.5

F32 = mybir.dt.float32
F32R = mybir.dt.float32r
EXP = mybir.ActivationFunctionType.Exp

_cache = {}


def _build():
    nc = bacc.Bacc(
        "TRN2",
        target_bir_lowering=False,
        debug=False,
        enable_asserts=False,
        num_devices=N_CORES,
    )

    q_d = nc.dram_tensor("q", [PER_CORE, I, D], F32, kind="ExternalInput").ap()
    kv_d = nc.dram_tensor("kv", [PER_CORE, J, D], F32, kind="ExternalInput").ap()
    a_d = nc.dram_tensor("A", [D, D], F32, kind="ExternalInput").ap()
    wvo_d = nc.dram_tensor("WvoT", [D, D], F32, kind="ExternalInput").ap()
    boutb_d = nc.dram_tensor("bout_b", [P, D], F32, kind="ExternalInput").ap()
    ident_d = nc.dram_tensor("ident", [P, P], F32, kind="ExternalInput").ap()
    out_d = nc.dram_tensor("out", [PER_CORE, I, D], F32, kind="ExternalOutput").ap()

    with tile.TileContext(nc) as tc, ExitStack() as ctx:
        const = ctx.enter_context(tc.tile_pool(name="const", bufs=1))
        qkv = ctx.enter_context(tc.tile_pool(name="qkv", bufs=2))
        tp = ctx.enter_context(tc.tile_pool(name="tp", bufs=2))
        ep = ctx.enter_context(tc.tile_pool(name="ep", bufs=4))
        fin = ctx.enter_context(tc.tile_pool(name="fin", bufs=2))
        ps = ctx.enter_context(tc.tile_pool(name="ps", bufs=8, space="PSUM"))

        # ---- constants (loaded / prepared once) ----
        ident = const.tile([P, P], F32, tag="ident")
        nc.sync.dma_start(ident[:], ident_d)
        a_f32 = const.tile([D, D], F32, tag="a_f32")
        nc.sync.dma_start(a_f32[:], a_d)
        wvo_f32 = const.tile([D, D], F32, tag="wvo_f32")
        nc.sync.dma_start(wvo_f32[:], wvo_d)
        bout_b = const.tile([P, D], F32, tag="bout_b")
        nc.sync.dma_start(bout_b[:], boutb_d)

        a_r = const.tile([D, D], F32R, tag="a_r")
        nc.vector.tensor_copy(a_r[:], a_f32[:])
        wvo_r = const.tile([D, D], F32R, tag="wvo_r")
        nc.vector.tensor_copy(wvo_r[:], wvo_f32[:])
        ones_f32 = const.tile([P, 1], F32, tag="ones_f32")
        nc.vector.memset(ones_f32[:], 1.0)
        ones_r = const.tile([P, 1], F32R, tag="ones_r")
        nc.vector.tensor_copy(ones_r[:], ones_f32[:])
        one1 = const.tile([1, 1], F32, tag="one1")
        nc.vector.memset(one1[:], 1.0)

        for bh in range(PER_CORE):
            # ---- load q, kv as [p, (tile, d)] ----
            q_sb = qkv.tile([P, I], F32, tag="q_sb")
            nc.sync.dma_start(
                q_sb[:].rearrange("p (t d) -> p t d", t=IT),
                q_d[bh].rearrange("(t p) d -> p t d", p=P),
            )
            kv_sb = qkv.tile([P, J], F32, tag="kv_sb")
            nc.sync.dma_start(
                kv_sb[:].rearrange("p (t d) -> p t d", t=JT),
                kv_d[bh].rearrange("(t p) d -> p t d", p=P),
            )

            # ---- transpose to [d, seq] (f32r), 4 tiles per PSUM buffer ----
            qT = tp.tile([P, I], F32R, tag="qT")
            kvT = tp.tile([P, J], F32R, tag="kvT")
            for dst, src, nt in ((qT, q_sb, IT), (kvT, kv_sb, JT)):
                for g in range(0, nt, 4):
                    pt = ps.tile([P, IC], F32, tag="ps")
                    for t in range(4):
                        nc.tensor.transpose(
                            pt[:, t * P : (t + 1) * P],
                            src[:, (g + t) * P : (g + t + 1) * P],
                            ident[:],
                        )
                    nc.vector.tensor_copy(dst[:, g * P : (g + 4) * P], pt[:])

            # ---- uT = (q @ A)^T = A^T-contract: lhsT=A [dq, dkv], rhs=qT ----
            uT = tp.tile([P, I], F32R, tag="uT")
            for c in range(NIC):
                pu = ps.tile([P, IC], F32, tag="ps")
                nc.tensor.matmul(
                    pu[:], a_r[:], qT[:, c * IC : (c + 1) * IC],
                    start=True, stop=True,
                )
                nc.vector.tensor_copy(uT[:, c * IC : (c + 1) * IC], pu[:])

            # ---- vproj[j, e] = kv @ WvoT, 4 j-tiles per PSUM buffer ----
            vproj = tp.tile([P, J], F32R, tag="vproj")
            for g in range(0, JT, 4):
                pv4 = ps.tile([P, IC], F32, tag="ps")
                for t in range(4):
                    nc.tensor.matmul(
                        pv4[:, t * P : (t + 1) * P],
                        kvT[:, (g + t) * P : (g + t + 1) * P],
                        wvo_r[:],
                        start=True, stop=True,
                    )
                nc.vector.tensor_copy(vproj[:, g * P : (g + 4) * P], pv4[:])

            # ---- main attention loop ----
            for c in range(NIC):
                u_slice = uT[:, c * IC : (c + 1) * IC]
                p_pv = ps.tile([P, IC], F32, tag="ps")     # PV accumulator [e, ic]
                p_dn = ps.tile([1, IC], F32, tag="ps")     # denominator [1, ic]
                for jt in range(JT):
                    kv_slice = kvT[:, jt * P : (jt + 1) * P]
                    p_s = ps.tile([P, IC], F32, tag="ps")
                    nc.tensor.matmul(p_s[:], kv_slice, u_slice, start=True, stop=True)
                    e_sb = ep.tile([P, IC], F32R, tag="e_sb")
                    nc.scalar.activation(e_sb[:], p_s[:], EXP, scale=SCALE)
                    nc.tensor.matmul(
                        p_pv[:], vproj[:, jt * P : (jt + 1) * P], e_sb[:],
                        start=(jt == 0), stop=(jt == JT - 1),
                    )
                    nc.tensor.matmul(
                        p_dn[:], ones_r[:], e_sb[:],
                        start=(jt == 0), stop=(jt == JT - 1),
                    )

                # ---- finalize chunk: recip denom, transpose PV, scale, bias ----
                pvT = fin.tile([P, IC], F32, tag="pvT")
                nc.vector.tensor_copy(pvT[:], p_pv[:])
                dn_sb = fin.tile([1, IC], F32, tag="dn_sb")
                nc.vector.tensor_copy(dn_sb[:], p_dn[:])

                p_dt = ps.tile([P, IC], F32, tag="ps")
                for t in range(IC // P):
                    nc.tensor.matmul(
                        p_dt[:, t : t + 1],
                        dn_sb[:, t * P : (t + 1) * P],
                        one1[:],
                        start=True, stop=True,
                    )
                recip = fin.tile([P, IC // P], F32, tag="recip")
                nc.vector.reciprocal(recip[:], p_dt[:, 0 : IC // P])

                p_o = ps.tile([P, IC], F32, tag="ps")
                out_sb = fin.tile([P, IC], F32, tag="out_sb")
                for t in range(IC // P):
                    nc.tensor.transpose(
                        p_o[:, t * P : (t + 1) * P],
                        pvT[:, t * P : (t + 1) * P],
                        ident[:],
                    )
                    nc.vector.tensor_scalar_mul(
                        out_sb[:, t * P : (t + 1) * P],
                        p_o[:, t * P : (t + 1) * P],
                        recip[:, t : t + 1],
                    )
                    nc.vector.tensor_add(
                        out_sb[:, t * P : (t + 1) * P],
                        out_sb[:, t * P : (t + 1) * P],
                        bout_b[:],
                    )
                nc.sync.dma_start(
                    out_d[bh, c * IC : (c + 1) * IC, :].rearrange(
                        "(t p) e -> p t e", p=P
                    ),
                    out_sb[:].rearrange("p (t e) -> p t e", t=IC // P),
                )

    nc.compile()
    return nc


def kernel(q, kv, Wq, Wkv, Wout, bout):
    if "nc" not in _cache:
        _cache["nc"] = _build()
    nc = _cache["nc"]

    Wk = Wkv[:D].astype(np.float64)
    Wv = Wkv[D:].astype(np.float64)
    A = (Wq.astype(np.float64).T @ Wk).astype(np.float32)
    WvoT = (Wv.T @ Wout.astype(np.float64).T).astype(np.float32)
    bout_b = np.broadcast_to(np.asarray(bout, np.float32), (P, D)).copy()
    ident = np.eye(P, dtype=np.float32)

    qf = np.ascontiguousarray(np.asarray(q, np.float32).reshape(BH, I, D))
    kvf = np.ascontiguousarray(np.asarray(kv, np.float32).reshape(BH, J, D))

    in_maps = []
    for c in range(N_CORES):
        sl = slice(c * PER_CORE, (c + 1) * PER_CORE)
        in_maps.append(
            {
                "q": np.ascontiguousarray(qf[sl]),
                "kv": np.ascontiguousarray(kvf[sl]),
                "A": A,
                "WvoT": WvoT,
                "bout_b": bout_b,
                "ident": ident,
            }
        )

    global _last_in_maps
    _last_in_maps = in_maps

    res = run_bass_kernel_spmd(nc, in_maps, core_ids=list(range(N_CORES)))
    out = np.concatenate([r["out"] for r in res.results], axis=0)
    return out.reshape(B, H, I, D)


_last_in_maps = None


# revision 5
# speedup vs baseline: 23.1866x; 23.1866x over previous
"""Trainium2 Bass kernel for nn_CrossAtten: cross-attention
out = softmax((q Wq^T)(kv Wk^T)^T / sqrt(D)) @ (kv Wv^T) @ Wout^T + bout

Shapes (hardcoded): q,kv [4,16,2048,128] fp32; Wq,Wout [128,128]; Wkv [256,128]; bout [128].
Sharding: batch*heads (64 pairs) split 8 per NeuronCore across 8 cores (pure data parallel).

Algebraic restructure (host-side weight folding):
  A    = Wq^T @ Wk          -> scores S = q A kv^T     (one projected tensor u = qA)
  WvoT = Wv^T @ Wout^T      -> PV matmul directly yields final projection (pre-bias)
Softmax is computed max-free (logits ~ N(0,1), |logit| < ~7, exp is safe in fp32),
with scores produced transposed [j, i] so the PV contraction (over j) needs no
transpose of the attention matrix; denominators come from a ones-vector matmul.
PE matmuls run in float32r (TF32-class: ~1.6e-4 rel err, full 1 cycle/row speed).
"""
import sys

if "/opt/trn_rl_repo" not in sys.path:
    sys.path.insert(0, "/opt/trn_rl_repo")

from contextlib import ExitStack

import numpy as np

import concourse.bacc as bacc
import concourse.tile as tile
import concourse.mybir as mybir
from concourse.bass_utils import run_bass_kernel_spmd

B, H, I, J, D = 4, 16, 2048, 2048, 128
BH = B * H
N_CORES = 8
PER_CORE = BH // N_CORES          # 8 (b,h) pairs per core
P = 128                           # partitions
IT = I // P                       # 16 i-tiles
JT = J // P                       # 16 j-tiles
IC = 512                          # i-chunk (columns per scores/PV matmul)
NIC = I // IC                     # 4 i-chunks
SCALE = D ** -0.5

F32 = mybir.dt.float32
F32R = mybir.dt.float32r
EXP = mybir.ActivationFunctionType.Exp

_cache = {}


def _build(repeat=1):
    nc = bacc.Bacc(
        "TRN2",
        target_bir_lowering=False,
        debug=False,
        enable_asserts=False,
        num_devices=N_CORES,
    )

    q_d = nc.dram_tensor("q", [PER_CORE, I, D], F32, kind="ExternalInput").ap()
    kv_d = nc.dram_tensor("kv", [PER_CORE, J, D], F32, kind="ExternalInput").ap()
    a_d = nc.dram_tensor("A", [D, D], F32, kind="ExternalInput").ap()
    wvo_d = nc.dram_tensor("WvoT", [D, D], F32, kind="ExternalInput").ap()
    boutb_d = nc.dram_tensor("bout_b", [P, D], F32, kind="ExternalInput").ap()
    ident_d = nc.dram_tensor("ident", [P, P], F32, kind="ExternalInput").ap()
    out_d = nc.dram_tensor("out", [PER_CORE, I, D], F32, kind="ExternalOutput").ap()

    with tile.TileContext(nc) as tc, ExitStack() as ctx:
        const = ctx.enter_context(tc.tile_pool(name="const", bufs=1))
        qkv = ctx.enter_context(tc.tile_pool(name="qkv", bufs=2))
        tp = ctx.enter_context(tc.tile_pool(name="tp", bufs=2))
        ep = ctx.enter_context(tc.tile_pool(name="ep", bufs=4))
        fin = ctx.enter_context(tc.tile_pool(name="fin", bufs=2))
        ps = ctx.enter_context(tc.tile_pool(name="ps", bufs=8, space="PSUM"))

        # ---- constants (loaded / prepared once) ----
        ident = const.tile([P, P], F32, tag="ident")
        nc.sync.dma_start(ident[:], ident_d)
        a_f32 = const.tile([D, D], F32, tag="a_f32")
        nc.sync.dma_start(a_f32[:], a_d)
        wvo_f32 = const.tile([D, D], F32, tag="wvo_f32")
        nc.sync.dma_start(wvo_f32[:], wvo_d)
        bout_b = const.tile([P, D], F32, tag="bout_b")
        nc.sync.dma_start(bout_b[:], boutb_d)

        a_r = const.tile([D, D], F32R, tag="a_r")
        nc.vector.tensor_copy(a_r[:], a_f32[:])
        wvo_r = const.tile([D, D], F32R, tag="wvo_r")
        nc.vector.tensor_copy(wvo_r[:], wvo_f32[:])
        ones_f32 = const.tile([P, 1], F32, tag="ones_f32")
        nc.vector.memset(ones_f32[:], 1.0)
        ones_r = const.tile([P, 1], F32R, tag="ones_r")
        nc.vector.tensor_copy(ones_r[:], ones_f32[:])
        one1 = const.tile([1, 1], F32, tag="one1")
        nc.vector.memset(one1[:], 1.0)

        for _rep, bh in [(r, b) for r in range(repeat) for b in range(PER_CORE)]:

            # ---- load q, kv as [p, (tile, d)] ----
            q_sb = qkv.tile([P, I], F32, tag="q_sb")
            nc.sync.dma_start(
                q_sb[:].rearrange("p (t d) -> p t d", t=IT),
                q_d[bh].rearrange("(t p) d -> p t d", p=P),
            )
            kv_sb = qkv.tile([P, J], F32, tag="kv_sb")
            nc.sync.dma_start(
                kv_sb[:].rearrange("p (t d) -> p t d", t=JT),
                kv_d[bh].rearrange("(t p) d -> p t d", p=P),
            )

            # ---- transpose to [d, seq] (f32r), 4 tiles per PSUM buffer ----
            qT = tp.tile([P, I], F32R, tag="qT")
            kvT = tp.tile([P, J], F32R, tag="kvT")
            for dst, src, nt in ((qT, q_sb, IT), (kvT, kv_sb, JT)):
                for g in range(0, nt, 4):
                    pt = ps.tile([P, IC], F32, tag="ps")
                    for t in range(4):
                        nc.tensor.transpose(
                            pt[:, t * P : (t + 1) * P],
                            src[:, (g + t) * P : (g + t + 1) * P],
                            ident[:],
                        )
                    nc.vector.tensor_copy(dst[:, g * P : (g + 4) * P], pt[:])

            # ---- uT = (q @ A)^T = A^T-contract: lhsT=A [dq, dkv], rhs=qT ----
            uT = tp.tile([P, I], F32R, tag="uT")
            for c in range(NIC):
                pu = ps.tile([P, IC], F32, tag="ps")
                nc.tensor.matmul(
                    pu[:], a_r[:], qT[:, c * IC : (c + 1) * IC],
                    start=True, stop=True,
                )
                nc.vector.tensor_copy(uT[:, c * IC : (c + 1) * IC], pu[:])

            # ---- vproj[j, e] = kv @ WvoT, 4 j-tiles per PSUM buffer ----
            vproj = tp.tile([P, J], F32R, tag="vproj")
            for g in range(0, JT, 4):
                pv4 = ps.tile([P, IC], F32, tag="ps")
                for t in range(4):
                    nc.tensor.matmul(
                        pv4[:, t * P : (t + 1) * P],
                        kvT[:, (g + t) * P : (g + t + 1) * P],
                        wvo_r[:],
                        start=True, stop=True,
                    )
                nc.vector.tensor_copy(vproj[:, g * P : (g + 4) * P], pv4[:])

            # ---- main attention loop ----
            for c in range(NIC):
                u_slice = uT[:, c * IC : (c + 1) * IC]
                p_pv = ps.tile([P, IC], F32, tag="ps")     # PV accumulator [e, ic]
                p_dn = ps.tile([1, IC], F32, tag="ps")     # denominator [1, ic]
                for jt in range(JT):
                    kv_slice = kvT[:, jt * P : (jt + 1) * P]
                    p_s = ps.tile([P, IC], F32, tag="ps")
                    nc.tensor.matmul(p_s[:], kv_slice, u_slice, start=True, stop=True)
                    e_sb = ep.tile([P, IC], F32R, tag="e_sb")
                    nc.scalar.activation(e_sb[:], p_s[:], EXP, scale=SCALE)
                    nc.tensor.matmul(
                        p_pv[:], vproj[:, jt * P : (jt + 1) * P], e_sb[:],
                        start=(jt == 0), stop=(jt == JT - 1),
                    )
                    nc.tensor.matmul(
                        p_dn[:], ones_r[:], e_sb[:],
                        start=(jt == 0), stop=(jt == JT - 1),
                    )

                # ---- finalize chunk: recip denom, transpose PV, scale, bias ----
                pvT = fin.tile([P, IC], F32, tag="pvT")
                nc.vector.tensor_copy(pvT[:], p_pv[:])
                dn_sb = fin.tile([1, IC], F32, tag="dn_sb")
                nc.vector.tensor_copy(dn_sb[:], p_dn[:])

                p_dt = ps.tile([P, IC], F32, tag="ps")
                for t in range(IC // P):
                    nc.tensor.matmul(
                        p_dt[:, t : t + 1],
                        dn_sb[:, t * P : (t + 1) * P],
                        one1[:],
                        start=True, stop=True,
                    )
                recip = fin.tile([P, IC // P], F32, tag="recip")
                nc.vector.reciprocal(recip[:], p_dt[:, 0 : IC // P])

                p_o = ps.tile([P, IC], F32, tag="ps")
                out_sb = fin.tile([P, IC], F32, tag="out_sb")
                for t in range(IC // P):
                    nc.tensor.transpose(
                        p_o[:, t * P : (t + 1) * P],
                        pvT[:, t * P : (t + 1) * P],
                        ident[:],
                    )
                    nc.vector.tensor_scalar_mul(
                        out_sb[:, t * P : (t + 1) * P],
                        p_o[:, t * P : (t + 1) * P],
                        recip[:, t : t + 1],
                    )
                    nc.vector.tensor_add(
                        out_sb[:, t * P : (t + 1) * P],
                        out_sb[:, t * P : (t + 1) * P],
                        bout_b[:],
                    )
                nc.sync.dma_start(
                    out_d[bh, c * IC : (c + 1) * IC, :].rearrange(
                        "(t p) e -> p t e", p=P
                    ),
                    out_sb[:].rearrange("p (t e) -> p t e", t=IC // P),
                )

    nc.compile()
    return nc


def kernel(q, kv, Wq, Wkv, Wout, bout):
    if "nc" not in _cache:
        _cache["nc"] = _build()
    nc = _cache["nc"]

    Wk = Wkv[:D].astype(np.float64)
    Wv = Wkv[D:].astype(np.float64)
    A = (Wq.astype(np.float64).T @ Wk).astype(np.float32)
    WvoT = (Wv.T @ Wout.astype(np.float64).T).astype(np.float32)
    bout_b = np.broadcast_to(np.asarray(bout, np.float32), (P, D)).copy()
    ident = np.eye(P, dtype=np.float32)

    qf = np.ascontiguousarray(np.asarray(q, np.float32).reshape(BH, I, D))
    kvf = np.ascontiguousarray(np.asarray(kv, np.float32).reshape(BH, J, D))

    in_maps = []
    for c in range(N_CORES):
        sl = slice(c * PER_CORE, (c + 1) * PER_CORE)
        in_maps.append(
            {
                "q": np.ascontiguousarray(qf[sl]),
                "kv": np.ascontiguousarray(kvf[sl]),
                "A": A,
                "WvoT": WvoT,
                "bout_b": bout_b,
                "ident": ident,
            }
        )

    global _last_in_maps
    _last_in_maps = in_maps

    res = run_bass_kernel_spmd(nc, in_maps, core_ids=list(range(N_CORES)))
    out = np.concatenate([r["out"] for r in res.results], axis=0)
    return out.reshape(B, H, I, D)


_last_in_maps = None
